# revision 62
# baseline (speedup 1.0000x reference)
"""MoE feed-forward (8 experts, hard argmin routing) on 8 TRN2 NeuronCores.

Strategy
--------
Host (numpy): rms_norm + argmin routing (0.13% of FLOPs), then a dispatch
plan: a single SPMD program whose per-core token slots are a short list of
ragged segment sizes (e.g. (488, 416, 136)) chosen by a small exact-cover
search so that all 8 cores carry ~1024 tokens (vs 1152 with uniform
128-tiles).  Every (core, segment) is bound to one expert; the host packs
that expert's weights and the segment's tokens.

Precision: bf16 with fp32 PSUM accumulation, except one quarter of the
up contraction (ko 6,7) and one eighth of the down contraction (kh 14,15)
which run as e4m3 DoubleRow pair-matmuls (2x PE throughput for those
slices).  Measured end-to-end max-rel error 1.66e-2 vs the 2e-2 gate,
bit-exact with the numpy simulation of the same quantization.

Device (Bass/Tile, SPMD x8): per segment, STREAM the expert weights
through one ring tile pool in exact consumption order (up8 pairs, up
q0..q3, down, dn8), each tile split into sub-DMAs so slices land via
parallel queues and become consumable one by one; ko is the outer loop
within each up half-round so the first matmul needs only one weight
slice.  up-proj (K=1024) -> swiglu (ACT Silu + DVE mul, e4m3 copies for
kh 14,15) -> down-proj (K=2048), yT written back to DRAM in bf16.  A
short dependency-free PE warm-up covers the initial DMA latency and
releases the HAM clock gate; dma_start dispatch (~640ns each on the sync
sequencer) is the start-latency bottleneck, so the first loads are split
finely and everything else is batched 1MB-per-tile.

Host: scatter y back to token order and add the skip connection in fp32.
"""

import functools
import json

import ml_dtypes
import numpy as np

N_EXPERTS = 8
DIM = 1024
HID = 2048
N_CORES = 8
P = 128
EPS = 1e-6

BF16 = ml_dtypes.bfloat16


# ----------------------------------------------------------------------------
# BIR fixup: walrus in this container accepts at most ONE sync-wait per
# instruction.  Split instructions with k>1 waits into (k-1) pure-wait
# EventSemaphore instructions on the same engine immediately before.
# ----------------------------------------------------------------------------
def _split_multiwait_json(bir_bytes: bytes) -> bytes:
    m = json.loads(bir_bytes)
    ctr = 0
    for func in m["functions"]:
        for bb in func["blocks"]:
            out = []
            for inst in bb["instructions"]:
                si = inst.get("sync_info")
                waits = (si or {}).get("on_wait") or []
                if len(waits) > 1:
                    for w in waits[:-1]:
                        ctr += 1
                        out.append({
                            "debug": inst.get("debug", 0),
                            "engine": inst["engine"],
                            "ins": [],
                            "outs": [],
                            "name": f"waitfix_{ctr}",
                            "opcode": "EventSemaphore",
                            "sync_info": {"on_update": [], "on_wait": [w]},
                        })
                    si["on_wait"] = [waits[-1]]
                out.append(inst)
            bb["instructions"] = out
    return json.dumps(m).encode()


def _patch_bass_json(nc):
    orig = nc.to_json_bytes

    def patched():
        return _split_multiwait_json(orig())

    nc.to_json_bytes = patched


# ----------------------------------------------------------------------------
# Host-side routing (replicates the reference numerics in fp32)
# ----------------------------------------------------------------------------
def _route(x, scale, centroids):
    xf = x.reshape(-1, DIM).astype(np.float32)
    ms = np.mean(xf * xf, axis=-1, keepdims=True)
    s = scale.astype(np.float32) / np.sqrt(ms + EPS)
    xn = xf * s
    nx = np.sum(xn * xn, axis=-1)[:, None]
    ny = np.sum(centroids * centroids, axis=-1)[None, :]
    d2 = nx + ny - 2.0 * (xn @ centroids.T)
    ids = np.argmin(d2, axis=-1).astype(np.int32)
    return xn, ids


# ----------------------------------------------------------------------------
# Dispatch planner: shared ragged segment-size list, exact-cover search
# ----------------------------------------------------------------------------
def _feasible_usage(sizes, t_e, node_budget=200_000):
    """Find u[e][j] (slots of size sizes[j] used by expert e) with
    column sums <= N_CORES and sum_j u[e][j]*sizes[j] >= t_e.  Experts are
    processed largest-first with a capacity prune; gives up (returns None)
    past node_budget.  Returns the usage matrix (original expert order)
    or None."""
    G = len(sizes)
    order = sorted(range(len(t_e)), key=lambda e: -t_e[e])
    ts = [t_e[e] for e in order]
    suffix = [0] * (len(ts) + 1)
    for i in range(len(ts) - 1, -1, -1):
        suffix[i] = suffix[i + 1] + ts[i]
    nodes = [0]

    @functools.lru_cache(maxsize=None)
    def rec(i, avail):
        if i == len(ts):
            return ()
        nodes[0] += 1
        if nodes[0] > node_budget:
            return None
        if sum(a * z for a, z in zip(avail, sizes)) < suffix[i]:
            return None
        need = ts[i]

        def gen(j, acc):
            if acc >= need:
                yield (0,) * (G - j)
                return
            if j == G:
                return
            for u in range(avail[j], -1, -1):
                for rest in gen(j + 1, acc + u * sizes[j]):
                    yield (u,) + rest

        for usage in gen(0, 0):
            na = tuple(a - u for a, u in zip(avail, usage))
            sub = rec(i + 1, na)
            if sub is not None:
                return (usage,) + sub
        return None

    res = rec(0, (N_CORES,) * G)
    if res is None:
        return None
    out = [None] * len(t_e)
    for i, e in enumerate(order):
        out[e] = res[i]
    return out


def _search_struct(t_e):
    """Smallest-cost shared segment-size list (descending) covering t_e."""
    total = sum(t_e)
    lo = (total + N_CORES - 1) // N_CORES

    # known-good structures for the expected routing distribution first
    # (473, 309, 245) is the minimal-S exact-cover-search result (S=1027)
    for struct in ((473, 309, 245), (488, 416, 136), (504, 416, 128)):
        if sum(struct) >= lo:
            u = _feasible_usage(struct, tuple(t_e))
            if u is not None:
                return struct, u

    # ladder search: smallest S first, modest candidate family per S
    S0 = ((lo + 7) // 8) * 8
    for S in range(S0, S0 + 161, 8):
        cands = set()
        for b in (512, 480, 448, 416, 384, 352, 320, 288, 256):
            for c in (128, 136, 144, 160, 192, 224, 256):
                a = S - b - c
                if 128 <= c <= b <= a <= 512:
                    cands.add((a, b, c))
        for struct in sorted(cands):
            u = _feasible_usage(struct, tuple(t_e), node_budget=50_000)
            if u is not None:
                return struct, u

    # guaranteed fallback: uniform 512 slots
    need = [(t + 511) // 512 for t in t_e]
    G = max(1, (sum(need) + N_CORES - 1) // N_CORES)
    while True:
        struct = (512,) * G
        u = _feasible_usage(struct, tuple(t_e), node_budget=2_000_000)
        if u is not None:
            return struct, u
        G += 1


def _plan(ids):
    tok_by_e = [np.where(ids == e)[0] for e in range(N_EXPERTS)]
    t_e = [len(t) for t in tok_by_e]
    struct, usage = _search_struct(t_e)
    G = len(struct)
    # deal expert slot-usages out to cores: per size-position j, a list of
    # experts occupying the 8 copies
    slot_experts = [[] for _ in range(G)]
    for e in range(N_EXPERTS):
        for j in range(G):
            slot_experts[j].extend([e] * usage[e][j])
    for j in range(G):
        slot_experts[j].extend([None] * (N_CORES - len(slot_experts[j])))
    # assignment: core c takes slot_experts[j][c]; cut tokens greedily
    assign = {}
    cursor = [0] * N_EXPERTS
    take = {}
    for j in range(G):
        for c in range(N_CORES):
            e = slot_experts[j][c]
            assign[(c, j)] = e
            if e is None:
                take[(c, j)] = 0
                continue
            t = min(struct[j], t_e[e] - cursor[e])
            t = max(t, 0)
            take[(c, j)] = t
            cursor[e] += t
    # leftover tokens (greedy cut order may strand some when a later slot
    # was filled before an earlier partial one) -- place into any slot of
    # the same expert with spare room
    for e in range(N_EXPERTS):
        rem = t_e[e] - cursor[e]
        if rem > 0:
            for (c, j), ee in assign.items():
                if ee == e and take[(c, j)] < struct[j]:
                    add = min(struct[j] - take[(c, j)], rem)
                    take[(c, j)] += add
                    cursor[e] += add
                    rem -= add
                    if rem == 0:
                        break
        assert rem == 0, "planner failed to place all tokens"
    return struct, assign, take, tok_by_e


# ----------------------------------------------------------------------------
# Device program
# ----------------------------------------------------------------------------
def _build_program(struct):
    import concourse.bass as bass
    import concourse.mybir as mybir
    import concourse.tile as tile

    f32 = mybir.dt.float32
    bf16 = mybir.dt.bfloat16
    f8e4 = mybir.dt.float8e4
    DR = mybir.MatmulPerfMode.DoubleRow
    Silu = mybir.ActivationFunctionType.Silu

    K = len(struct)
    T = sum(struct)  # token slots per core

    nc = bass.Bass("TRN2", debug=False)
    # bf16 activations for up-contraction blocks ko 0..5
    xnt_in = nc.dram_tensor("xnt", [P, 6, T], bf16, kind="ExternalInput").ap()
    # e4m3 activation pairs (ko 6, ko 7) for the DoubleRow quarter of the
    # up contraction: [P, j=2, T]
    xnt8_in = nc.dram_tensor("xnt8", [P, 2, T], f8e4,
                             kind="ExternalInput").ap()
    # up weights bf16: per (segment, j-quad q of 4, ko-quad kq of 2):
    # [128, 4, 1024] with cols [a(4q)..a(4q+3) | g(4q)..g(4q+3)] per ko.
    # kq1 carries only koi 0,1 (= ko 4,5); ko 6,7 live in up8.
    up_in = nc.dram_tensor("up", [K, 4, 2, P, 4, 1024], bf16,
                           kind="ExternalInput").ap()
    # up weights e4m3 DoubleRow pairs: [segment, P, 2q+j, 1024]
    # (j = pair element: ko6, ko7)
    up8_in = nc.dram_tensor("up8", [K, P, 8, 1024], f8e4,
                            kind="ExternalInput").ap()
    # down weights: per (segment, kh-quad kq of 4): [128, 4, 1024]
    # (1024 = all 8 dout tiles) per kh.  kq3 (kh 12..15) lives in dn8 as
    # two e4m3 DoubleRow pairs (12,13) and (14,15).
    down_in = nc.dram_tensor("down", [K, 4, P, 4, 1024], bf16,
                             kind="ExternalInput").ap()
    dn8_in = nc.dram_tensor("dn8", [K, P, 4, 1024], f8e4,
                            kind="ExternalInput").ap()
    yt_out = nc.dram_tensor("yt", [P, 8, T], bf16, kind="ExternalOutput").ap()

    with tile.TileContext(nc) as tc:
        with (
            tc.tile_pool(name="wpool", bufs=14) as w_pool,
            tc.tile_pool(name="upf", bufs=8) as upf_pool,
            tc.tile_pool(name="xn", bufs=2) as xn_pool,
            tc.tile_pool(name="xn8", bufs=2) as xn8_pool,
            tc.tile_pool(name="xnf", bufs=4) as xnf_pool,
            tc.tile_pool(name="act", bufs=2) as act_pool,
            tc.tile_pool(name="act8", bufs=2) as act8_pool,
            tc.tile_pool(name="yc", bufs=2) as yc_pool,
            tc.tile_pool(name="ps", bufs=8, space="PSUM") as ps,
        ):
            # PE warm-up: dependency-free matmuls on an (uninitialized)
            # scratch tile keep PE busy while the first DMAs land (HAM
            # clock-gate release + hide DMA latency).
            with tc.tile_pool(name="warm", bufs=1) as warm_pool:
                wsrc = warm_pool.tile([P, 256], bf16, tag="warm")
                nc.gpsimd.memset(wsrc[:], 0.0)
                wps = [ps.tile([P, P], f32, tag="ps", name=f"wps{i}")
                       for i in range(2)]
                for i in range(26):
                    nc.tensor.matmul(wps[i % 2][:], wsrc[:, 0:P],
                                     wsrc[:, P : 2 * P],
                                     start=True, stop=True)

            # NOTE: spreading dma_start dispatch to other engines does not
            # help: gpsimd DMAs pin to one SWDGE queue, and scalar-issued
            # DMAs crashed the device (NRT_EXEC_UNIT_UNRECOVERABLE).
            def dma_spread(dst, src):
                nc.sync.dma_start(dst, src)

            col = 0
            for s in range(K):
                gn = struct[s]
                # e4m3 activation pairs for the DoubleRow part (tile free
                # dim padded to 512 so the pair-dim step stays 16B-aligned);
                # DMA emitted after the fine loads below (dispatch order).
                xn8_seg = xn8_pool.tile([P, 2, 512], f8e4, tag="xn8",
                                        name=f"xn8_{s}")
                if s == 0:
                    # first segment: finest-grained loads so the first
                    # matmuls' inputs (xn ko0, up q0 ko0 a/g quarters)
                    # arrive within ~2us of DMA start.  (ko, kq) source
                    # index: ko 0..3 -> kq0 koi=ko; ko 4,5 -> kq1 koi-4.
                    xn_parts = [None] * 3
                    up_fine0 = [None] * 6

                    def _upf_load(ko, splits):
                        wf = upf_pool.tile([P, 1, 1024], bf16, tag="upf",
                                           name=f"upf_{ko}")
                        kq, koi = (0, ko) if ko < 4 else (1, ko - 4)
                        w = 1024 // splits
                        # order quarters a-first/g-first interleaved:
                        # [0:256] (a sub0-1), [512:768] (g sub0-1), rest
                        order = range(splits) if splits <= 2 else (0, 2, 1, 3)
                        for i in order:
                            dma_spread(
                                wf[:, 0, i * w : (i + 1) * w],
                                up_in[0, 0, kq, :, koi, i * w : (i + 1) * w])
                        up_fine0[ko] = wf

                    def _xnf_load(kp, splits):
                        xk = xnf_pool.tile([P, 2, gn], bf16, tag="xnf",
                                           name=f"xn0_{kp}")
                        for i in range(splits):
                            h = 2 // splits
                            dma_spread(
                                xk[:, i * h : i * h + h, :],
                                xnt_in[:, 2 * kp + i * h : 2 * kp + i * h + h,
                                       0:gn])
                        xn_parts[kp] = xk

                    _xnf_load(0, 2)
                    _upf_load(0, 4)
                    _upf_load(1, 2)
                    _xnf_load(1, 2)
                    _upf_load(2, 1)
                    # q0's DoubleRow closer needs up8 pair-slice 0 by the
                    # end of the first half-round (~14.5us): dispatch its
                    # a-half and g-half here, ahead of ko3..5
                    up8_seg0 = w_pool.tile([P, 8, 1024], f8e4, tag="wpool",
                                           name="up8_0")
                    for h in range(2):
                        nc.sync.dma_start(
                            up8_seg0[:, 0:2, 512 * h : 512 * h + 512],
                            up8_in[0, :, 0:2, 512 * h : 512 * h + 512])
                    _upf_load(3, 1)
                    _xnf_load(2, 1)
                    _upf_load(4, 1)
                    _upf_load(5, 1)

                    def get_xn(ko, xn_parts=xn_parts):
                        return xn_parts[ko // 2][:, ko % 2, :]
                else:
                    xn_seg = xn_pool.tile([P, 6, gn], bf16, tag="xn",
                                          name=f"xn{s}")
                    for kp in range(3):
                        nc.sync.dma_start(
                            xn_seg[:, 2 * kp : 2 * kp + 2, :],
                            xnt_in[:, 2 * kp : 2 * kp + 2, col : col + gn])

                    def get_xn(ko, xn_seg=xn_seg):
                        return xn_seg[:, ko, :]
                # weights stream through one ring pool in exact consumption
                # order (up8 pairs, up q..., then down); tiles split into
                # sub-DMAs so they land via parallel queues and their ko
                # slices become consumable one by one.
                if s == 0:
                    up8_seg = up8_seg0  # q0 pair already loading
                    i0 = 1
                else:
                    up8_seg = w_pool.tile([P, 8, 1024], f8e4, tag="wpool",
                                          name=f"up8_{s}")
                    i0 = 0
                nc.sync.dma_start(xn8_seg[:, :, 0:gn],
                                  xnt8_in[:, :, col : col + gn])
                for i in range(i0, 4):
                    nc.sync.dma_start(up8_seg[:, 2 * i : 2 * i + 2, :],
                                      up8_in[s, :, 2 * i : 2 * i + 2, :])
                upt = {}
                for q in range(4):
                    if s == 0 and q == 0:
                        continue  # ko 0..5 loaded fine above
                    for kq in range(2):
                        w = w_pool.tile([P, 4, 1024], bf16, tag="wpool",
                                        name=f"up_{s}_{q}_{kq}")
                        nko = 4 if kq == 0 else 2  # kq1: only ko 4,5
                        for i in range(nko):
                            nc.sync.dma_start(w[:, i : i + 1, :],
                                              up_in[s, q, kq, :, i : i + 1, :])
                        upt[(q, kq)] = w

                def get_up(s_, q, ko, upt=upt, up_fine=up_fine0):
                    if s_ == 0 and q == 0:
                        return up_fine[ko][:, 0, :]
                    return upt[(q, ko // 4)][:, ko % 4, :]

                dnt = {}
                for kq in range(3):
                    w = w_pool.tile([P, 4, 1024], bf16, tag="wpool",
                                    name=f"dn_{s}_{kq}")
                    for i in range(4):
                        nc.sync.dma_start(w[:, i : i + 1, :],
                                          down_in[s, kq, :, i : i + 1, :])
                    dnt[kq] = w
                dn8_seg = w_pool.tile([P, 4, 1024], f8e4, tag="wpool",
                                      name=f"dn8_{s}")
                for i in range(2):
                    nc.sync.dma_start(dn8_seg[:, 2 * i : 2 * i + 2, :],
                                      dn8_in[s, :, 2 * i : 2 * i + 2, :])

                act_t = act_pool.tile([P, 16, gn], bf16, tag="act")
                # e4m3 copies of hidden blocks kh 12..15 (pair layout for
                # the down DoubleRow matmuls; free dim padded to 512 for
                # step alignment)
                act8_t = act8_pool.tile([P, 4, 512], f8e4, tag="act8")
                # ---- up projection: 4 quads x 2 half-rounds (2 subs) ----
                # ko is the OUTER loop within a half-round, so each weight
                # slice [P, 1024] is fully consumed before the next is
                # needed (smooth DMA demand).  ko 0..5 run bf16; ko 6,7 are
                # one e4m3 DoubleRow matmul (K=256 pairs) closing the
                # accumulation -- 2x PE throughput for that quarter.
                xr8 = xn8_seg[:, :, 0:gn]
                for q in range(4):
                    for half in range(2):
                        subs = (2 * half, 2 * half + 1)
                        pa = {sub: ps.tile([P, gn], f32, tag="ps",
                                           name=f"pa{sub}")
                              for sub in subs}
                        pg = {sub: ps.tile([P, gn], f32, tag="ps",
                                           name=f"pg{sub}")
                              for sub in subs}
                        for ko in range(6):
                            w = get_up(s, q, ko)
                            xr = get_xn(ko)
                            first = ko == 0
                            for sub in subs:
                                ca = sub * P
                                cg = 512 + sub * P
                                nc.tensor.matmul(pa[sub][:], w[:, ca : ca + P],
                                                 xr, start=first, stop=False)
                                nc.tensor.matmul(pg[sub][:], w[:, cg : cg + P],
                                                 xr, start=first, stop=False)
                        w8 = up8_seg[:, 2 * q : 2 * q + 2, :]
                        for sub in subs:
                            ca = sub * P
                            cg = 512 + sub * P
                            nc.tensor.matmul(pa[sub][:], w8[:, :, ca : ca + P],
                                             xr8, start=False, stop=True,
                                             perf_mode=DR)
                            nc.tensor.matmul(pg[sub][:], w8[:, :, cg : cg + P],
                                             xr8, start=False, stop=True,
                                             perf_mode=DR)
                        for sub in subs:
                            j = 4 * q + sub
                            nc.scalar.activation(act_t[:, j, :], pg[sub][:],
                                                 Silu)
                            if j >= 12:  # kh 12..15 -> e4m3 for down DR
                                nc.vector.tensor_mul(
                                    act8_t[:, j - 12, 0:gn], pa[sub][:],
                                    act_t[:, j, :])
                            else:
                                nc.vector.tensor_mul(act_t[:, j, :],
                                                     pa[sub][:],
                                                     act_t[:, j, :])
                # ---- down projection: 4 rounds x (12 kh bf16 + 2 DR) ----
                yc = yc_pool.tile([P, 8, gn], bf16, tag="yc", name=f"yc{s}")
                for rr in range(4):
                    pd = [ps.tile([P, gn], f32, tag="ps", name=f"pd{q}")
                          for q in range(2)]
                    for kh in range(12):
                        w = dnt[kh // 4][:, kh % 4, :]
                        first = kh == 0
                        for q in range(2):
                            c = (2 * rr + q) * P
                            nc.tensor.matmul(pd[q][:], w[:, c : c + P],
                                             act_t[:, kh, :],
                                             start=first, stop=False)
                    for pp in range(2):
                        for q in range(2):
                            c = (2 * rr + q) * P
                            nc.tensor.matmul(
                                pd[q][:],
                                dn8_seg[:, 2 * pp : 2 * pp + 2, c : c + P],
                                act8_t[:, 2 * pp : 2 * pp + 2, 0:gn],
                                start=False, stop=pp == 1,
                                perf_mode=DR)
                    for q in range(2):
                        nc.vector.tensor_copy(yc[:, 2 * rr + q, :], pd[q][:])
                    # per-round output DMA overlaps later rounds' matmuls
                    nc.sync.dma_start(
                        yt_out[:, 2 * rr : 2 * rr + 2, col : col + gn],
                        yc[:, 2 * rr : 2 * rr + 2, :])
                col += gn

    _patch_bass_json(nc)
    return nc


# ----------------------------------------------------------------------------
# Host-side weight packing into the streaming layouts
# ----------------------------------------------------------------------------
E4M3 = ml_dtypes.float8_e4m3fn  # matches TRN fp8e4 bit-for-bit below 240


def _pack_up(up_e_f32):
    """[DIM, 2H] f32 -> (bf16 [4 q, 2 kq, 128, 4 koi, 1024] for ko 0..5,
    e4m3 [128, 2q+j, 1024] DoubleRow pairs for ko 6,7)."""
    U = up_e_f32.reshape(8, P, 2 * HID)
    A = U[:, :, :HID].reshape(8, P, 16, P)
    G = U[:, :, HID:].reshape(8, P, 16, P)
    out = np.empty((4, 8, P, 1024), dtype=np.float32)
    for q in range(4):
        for i in range(4):
            out[q, :, :, i * P : (i + 1) * P] = A[:, :, 4 * q + i]
            out[q, :, :, 512 + i * P : 512 + (i + 1) * P] = G[:, :, 4 * q + i]
    # bf16 part: [4, 8ko, P, 1024] -> [4, 2, 4, P, 1024] -> [4, 2, P, 4, 1024]
    pack_bf = np.ascontiguousarray(
        out.astype(BF16).reshape(4, 2, 4, P, 1024).transpose(0, 1, 3, 2, 4)
    )
    # e4m3 pairs: up8[p, 2q+j, col] = out[q, 6+j, p, col]
    pack_f8 = np.ascontiguousarray(
        np.clip(out[:, 6:8], -240, 240).transpose(2, 0, 1, 3).reshape(P, 8, 1024)
    ).astype(E4M3)
    return pack_bf, pack_f8


def _pack_down(down_e_f32):
    """[HID, DIM] f32 -> (bf16 [4 kq, 128, 4 khi, 1024] for kh 0..11,
    e4m3 [128, 4, 1024] DoubleRow pairs (12,13),(14,15))."""
    D = down_e_f32.reshape(4, 4, P, DIM)
    pack_bf = np.ascontiguousarray(D.transpose(0, 2, 1, 3)).astype(BF16)
    pack_f8 = np.ascontiguousarray(
        np.clip(D[3], -240, 240).transpose(1, 0, 2)
    ).astype(E4M3)  # [P, khi=2pp+j, 1024]
    return pack_bf, pack_f8


# ----------------------------------------------------------------------------
# Entry point
# ----------------------------------------------------------------------------
def _run(inputs, trace=False, tmpdir=None):
    from concourse.bass_utils import run_bass_kernel_spmd

    x = np.asarray(inputs["x"])
    scale = np.asarray(inputs["scale"])
    centroids = np.asarray(inputs["centroids"])
    up_w = np.asarray(inputs["up_w"])
    down_w = np.asarray(inputs["down_w"])

    B, S, D = x.shape
    ntok = B * S
    xf32 = x.reshape(ntok, D).astype(np.float32)

    xn, ids = _route(x, scale, centroids)
    struct, assign, take, tok_by_e = _plan(ids)
    K = len(struct)
    T = sum(struct)
    offs = np.concatenate([[0], np.cumsum(struct)])

    # pre-pack each expert's weights once (experts can appear on many cores)
    up_packed_e = {}
    up8_packed_e = {}
    down_packed_e = {}
    dn8_packed_e = {}
    for e in range(N_EXPERTS):
        if any(v == e for v in assign.values()):
            up_packed_e[e], up8_packed_e[e] = _pack_up(
                up_w[e].astype(np.float32))
            down_packed_e[e], dn8_packed_e[e] = _pack_down(
                down_w[e].astype(np.float32))

    xnT = np.ascontiguousarray(xn.T)  # [DIM, ntok] f32
    cursor = [0] * N_EXPERTS
    core_cols_tok = [np.zeros(T, dtype=np.int64) for _ in range(N_CORES)]
    core_cols_valid = [np.zeros(T, dtype=bool) for _ in range(N_CORES)]
    in_maps = []
    for c in range(N_CORES):
        up_pack = np.zeros((K, 4, 2, P, 4, 1024), dtype=BF16)
        up8_pack = np.zeros((K, P, 8, 1024), dtype=E4M3)
        down_pack = np.zeros((K, 4, P, 4, 1024), dtype=BF16)
        dn8_pack = np.zeros((K, P, 4, 1024), dtype=E4M3)
        for j in range(K):
            e = assign.get((c, j))
            if e is None:
                continue
            up_pack[j] = up_packed_e[e]
            up8_pack[j] = up8_packed_e[e]
            down_pack[j] = down_packed_e[e]
            dn8_pack[j] = dn8_packed_e[e]
            t = take[(c, j)]
            if t:
                sel = tok_by_e[e][cursor[e] : cursor[e] + t]
                cursor[e] += t
                core_cols_tok[c][offs[j] : offs[j] + t] = sel
                core_cols_valid[c][offs[j] : offs[j] + t] = True
        xnt_cols = xnT[:, core_cols_tok[c]]  # [DIM, T] f32
        xnt_pack = np.ascontiguousarray(
            xnt_cols[: 6 * P].astype(BF16).reshape(6, P, T).transpose(1, 0, 2)
        )  # [P, 6, T] bf16 (ko 0..5)
        xnt8_pack = np.ascontiguousarray(
            np.clip(xnt_cols[6 * P :], -240, 240)
            .reshape(2, P, T).transpose(1, 0, 2)
        ).astype(E4M3)  # [P, 2, T] e4m3 pairs (ko 6,7)
        in_maps.append({"xnt": xnt_pack, "xnt8": xnt8_pack,
                        "up": up_pack, "up8": up8_pack,
                        "down": down_pack, "dn8": dn8_pack})

    for e in range(N_EXPERTS):
        assert cursor[e] == len(tok_by_e[e]), "dispatch did not cover all tokens"

    nc = _build_program(struct)
    kwargs = {}
    if trace:
        kwargs = dict(trace=True, tmpdir=tmpdir)
    res = run_bass_kernel_spmd(nc, in_maps, core_ids=list(range(N_CORES)), **kwargs)

    # ---- scatter + skip ----
    out = xf32.copy()
    for c in range(N_CORES):
        # yt_out layout is [P, 8 dout-tiles, T]; dout index = do*128 + p
        yt = np.ascontiguousarray(
            res.results[c]["yt"].reshape(P, 8, T).transpose(1, 0, 2)
        ).reshape(8 * P, T).astype(np.float32)  # [DIM, T]
        valid = core_cols_valid[c]
        toks = core_cols_tok[c][valid]
        out[toks] = xf32[toks] + yt[:, valid].T
    return out.reshape(B, S, D).astype(x.dtype), res


def kernel(**inputs) -> np.ndarray:
    out, _ = _run(inputs)
    return out
